# revision 9
# baseline (speedup 1.0000x reference)
"""NMI loss v2: engine-balanced Gaussian soft-histogram on 8 trn2 cores.

Per core: N = 262144 voxels as [128 part, 2048 cols], chunks of VC=512 cols,
double-buffered bin-major I-tile pairs [128, 33, VC] (u16 raw; fp16 y view,
bf16 I view, in-place strided ACT exp).

y = (31*a - k)^2 per bin k:
  a-side (32 bins, DVE): TS d=31*ah-k (fp16 4x) -> TT d*d (2x)
         -> TT += 0.5*ln(S_a) bcast (2x)   [folds 1/S_a into the exp]
  b-side: bins [0,BSPLIT) DVE, bins [BSPLIT,32) ACT Square(31*bh - k).
  exp: ACT Exp(scale=-2) in place fp16->bf16.
S(t) analytic: 1.2533141*(1+0.01441324*cos(2pi t)) - e^{-2(t+1)^2}
  - e^{-2(32-t)^2}; cos via range-reduced ACT Sin (fp16 magic rounding).
Gram: lhsT=[I_an | inv_sb], rhs=[I_b | 1]; 2048 strided 33-col matmuls into
one [33,33] f32 PSUM; host sums 8 core stats + log-MI.
Inputs shipped fp16 (input quantization = position jitter, averages out).
"""

import sys
import numpy as np

sys.path.insert(0, "/opt/trn_rl_repo")

NCORES = 8
P = 128
B = 32
S = B + 1
NVOX_TOTAL = 128 ** 3
NVOX = NVOX_TOTAL // NCORES
COLS = NVOX // P            # 2048
VC = 512
NCH = COLS // VC            # 4
BSPLIT = 20
NBANK = 8

_CACHE = {}

PI = float(np.pi)
C_P0 = 1.2533141373155003
C_P1 = 0.014413237061177604 * C_P0


def _build_nc():
    from contextlib import ExitStack
    from concourse import bass, mybir

    f32 = mybir.dt.float32
    fp16 = mybir.dt.float16
    bf16 = mybir.dt.bfloat16
    u16 = mybir.dt.uint16
    AF = mybir.ActivationFunctionType
    A = mybir.AluOpType

    nc = bass.Bass()
    a_d = nc.dram_tensor("a", [P, COLS], fp16, kind="ExternalInput")
    b_d = nc.dram_tensor("b", [P, COLS], fp16, kind="ExternalInput")
    out_d = nc.dram_tensor("stats", [S, NBANK * S], f32, kind="ExternalOutput")

    with ExitStack() as ctx:
        e = ctx.enter_context
        ah = e(nc.sbuf_tensor("ah", [P, COLS], fp16))
        bh = e(nc.sbuf_tensor("bh", [P, COLS], fp16))
        ia = [e(nc.sbuf_tensor(f"ia{i}", [P, S * VC], u16)) for i in range(2)]
        ib = [e(nc.sbuf_tensor(f"ib{i}", [P, S * VC], u16)) for i in range(2)]
        th = e(nc.sbuf_tensor("th", [P, COLS], fp16))
        tf = e(nc.sbuf_tensor("tf", [P, COLS], f32))
        cs = e(nc.sbuf_tensor("cs", [P, COLS], f32))
        ssum = e(nc.sbuf_tensor("ssum", [P, COLS], f32))
        mya = e(nc.sbuf_tensor("mya", [P, COLS], fp16))
        invb = e(nc.sbuf_tensor("invb", [P, COLS], bf16))
        bias_pi2 = e(nc.sbuf_tensor("bias_pi2", [P, 1], f32))
        bias_m32 = e(nc.sbuf_tensor("bias_m32", [P, 1], f32))
        bias_k = [
            e(nc.sbuf_tensor(f"bias_k{kb}", [P, 1], f32))
            for kb in range(BSPLIT, B)
        ]
        stats_sb = e(nc.sbuf_tensor("stats_sb", [S, NBANK * S], f32))
        acc = [e(nc.psum_tensor(f"acc_ps{i}", [S, S], f32)) for i in range(NBANK)]

        s_in = e(nc.semaphore("s_in"))
        s_in_b = e(nc.semaphore("s_in_b"))
        s_pre = e(nc.semaphore("s_pre"))
        s_va = e(nc.semaphore("s_va"))
        s_sa = e(nc.semaphore("s_sa"))
        s_bld_a = e(nc.semaphore("s_bld_a"))
        s_bld_b = e(nc.semaphore("s_bld_b"))
        s_exp_a = e(nc.semaphore("s_exp_a"))
        s_exp_b = e(nc.semaphore("s_exp_b"))
        s_row = e(nc.semaphore("s_row"))
        s_pe = e(nc.semaphore("s_pe"))
        s_done = e(nc.semaphore("s_done"))
        s_out = e(nc.semaphore("s_out"))
        block = e(nc.Block())

        def i3(buf, dt):
            return buf[:, :].rearrange("p (s n) -> p s n", s=S).bitcast(dt)

        @block.sync
        def _(sync):
            sync.dma_start(ah[:, :], a_d[:, :]).then_inc(s_in, 16)
            sync.dma_start(bh[:, :], b_d[:, :]).then_inc(s_in_b, 16)

        @block.gpsimd
        def _(g):
            g.memset(bias_pi2[:, :], PI / 2)
            g.memset(bias_m32[:, :], -32.0)
            for i, kb in enumerate(range(BSPLIT, B)):
                g.memset(bias_k[i][:, :], -float(kb))
            for i in range(2):
                g.memset(i3(ib[i], bf16)[:, B, :], 1.0).then_inc(s_pre, 1)
            g.wait_ge(s_done, 1)
            g.dma_start(out_d[:, :], stats_sb[:, :]).then_inc(s_out, 16)
            g.wait_ge(s_out, 16)

        @block.vector
        def _(v):
            def sphase_head(x):
                v.tensor_scalar(
                    out=th[:, :], in0=x[:, :], scalar1=31.0, scalar2=1536.0,
                    op0=A.mult, op1=A.add,
                )
                v.tensor_scalar(
                    out=tf[:, :], in0=x[:, :], scalar1=31.0, scalar2=None,
                    op0=A.mult,
                )
                v.scalar_tensor_tensor(
                    out=cs[:, :], in0=th[:, :], scalar=-1536.0, in1=tf[:, :],
                    op0=A.add, op1=A.subtract,
                ).then_inc(s_va, 1)

            def sphase_tail(base_s):
                v.wait_ge(s_sa, base_s + 1)
                v.tensor_scalar(
                    out=ssum[:, :], in0=cs[:, :], scalar1=C_P1, scalar2=C_P0,
                    op0=A.mult, op1=A.add,
                )
                v.wait_ge(s_sa, base_s + 3)
                v.tensor_tensor(
                    ssum[:, :], ssum[:, :], tf[:, :], op=A.subtract
                ).then_inc(s_va, 1)
                v.wait_ge(s_sa, base_s + 5)
                v.tensor_tensor(
                    ssum[:, :], ssum[:, :], tf[:, :], op=A.subtract
                ).then_inc(s_va, 1)

            def build_a(c):
                k = c % 2
                iaf = i3(ia[k], fp16)
                asl = ah[:, c * VC : (c + 1) * VC]
                for kb in range(B):
                    v.tensor_scalar(
                        out=iaf[:, kb, :], in0=asl, scalar1=31.0,
                        scalar2=-float(kb), op0=A.mult, op1=A.add,
                    )
                da = iaf[:, 0:B, :]
                v.tensor_tensor(da, da, da, op=A.mult)

            def build_b(c):
                k = c % 2
                ibf = i3(ib[k], fp16)
                bsl = bh[:, c * VC : (c + 1) * VC]
                for kb in range(BSPLIT):
                    v.tensor_scalar(
                        out=ibf[:, kb, :], in0=bsl, scalar1=31.0,
                        scalar2=-float(kb), op0=A.mult, op1=A.add,
                    )
                db = ibf[:, 0:BSPLIT, :]
                v.tensor_tensor(db, db, db, op=A.mult).then_inc(s_bld_b, 1)

            def add_m(c):
                k = c % 2
                da = i3(ia[k], fp16)[:, 0:B, :]
                msl = (
                    mya[:, c * VC : (c + 1) * VC]
                    .rearrange("p (o n) -> p o n", o=1)
                    .broadcast_to([P, B, VC])
                )
                v.tensor_tensor(da, da, msl, op=A.add).then_inc(s_bld_a, 1)

            def row_b(c):
                k = c % 2
                v.tensor_copy(
                    i3(ia[k], bf16)[:, B, :], invb[:, c * VC : (c + 1) * VC]
                ).then_inc(s_row, 1)

            v.wait_ge(s_in, 16)
            sphase_head(ah)          # s_va 1
            build_a(0)               # prebuild chunk0 a (raw d^2)
            v.wait_ge(s_in_b, 16)
            build_b(0)               # s_bld_b 1
            sphase_tail(0)           # s_va 2,3
            v.wait_ge(s_sa, 6)       # Ln_a done
            v.tensor_scalar(
                out=mya[:, :], in0=cs[:, :], scalar1=0.5, scalar2=None,
                op0=A.mult,
            )
            sphase_head(bh)          # s_va 4
            add_m(0)                 # s_bld_a 1  -> exp_a(c0) can go
            sphase_tail(6)           # s_va 5,6
            v.wait_ge(s_sa, 13)      # invb (= exp(-ln S_b)) ready
            row_b(0)                 # s_row 1 -> PE chunk0 can go
            build_a(1)
            add_m(1)                 # s_bld_a 2
            row_b(1)                 # s_row 2
            build_b(1)               # s_bld_b 2
            for c in range(2, NCH):
                v.wait_ge(s_pe, c - 1)
                build_a(c)
                add_m(c)             # s_bld_a c+1
                row_b(c)             # s_row c+1
                build_b(c)           # s_bld_b c+1

            v.wait_ge(s_pe, NCH)
            for i in range(NBANK):
                mm = v.tensor_copy(
                    stats_sb[:, i * S : (i + 1) * S], acc[i][:, :]
                )
            mm.then_inc(s_done, 1)

        @block.scalar
        def _(sc):
            def bsq(c):
                k = c % 2
                ibf = i3(ib[k], fp16)
                bsl = bh[:, c * VC : (c + 1) * VC]
                for i, kb in enumerate(range(BSPLIT, B)):
                    sc.activation(
                        ibf[:, kb, :], bsl, AF.Square, scale=31.0,
                        bias=bias_k[i][:, :],
                    )

            def schain(x, base_v, base_s):
                sc.wait_ge(s_va, base_v + 1)
                sc.activation(
                    cs[:, :], cs[:, :], AF.Sin, scale=-2.0 * PI,
                    bias=bias_pi2[:, :],
                ).then_inc(s_sa, 1)
                sc.activation(
                    tf[:, :], x[:, :], AF.Square, scale=31.0, bias=1.0
                ).then_inc(s_sa, 1)
                sc.activation(
                    tf[:, :], tf[:, :], AF.Exp, scale=-2.0
                ).then_inc(s_sa, 1)
                sc.wait_ge(s_va, base_v + 2)
                sc.activation(
                    tf[:, :], x[:, :], AF.Square, scale=31.0,
                    bias=bias_m32[:, :],
                ).then_inc(s_sa, 1)
                sc.activation(
                    tf[:, :], tf[:, :], AF.Exp, scale=-2.0
                ).then_inc(s_sa, 1)

            NQ = 8

            def exps(c):
                k = c % 2
                iaf = i3(ia[k], fp16)
                ibf = i3(ib[k], fp16)
                H = VC // NQ
                sc.wait_ge(s_bld_a, c + 1)
                sc.wait_ge(s_bld_b, c + 1)
                for h in range(NQ):
                    sc.activation(
                        i3(ia[k], bf16)[:, 0:B, h * H : (h + 1) * H],
                        iaf[:, 0:B, h * H : (h + 1) * H], AF.Exp,
                        scale=-2.0,
                    ).then_inc(s_exp_a, 1)
                    sc.activation(
                        i3(ib[k], bf16)[:, 0:B, h * H : (h + 1) * H],
                        ibf[:, 0:B, h * H : (h + 1) * H], AF.Exp,
                        scale=-2.0,
                    ).then_inc(s_exp_b, 1)

            sc.wait_ge(s_in, 16)
            sc.wait_ge(s_in_b, 16)
            sc.wait_ge(s_pre, 2)
            bsq(0)
            schain(ah, 0, 0)
            sc.wait_ge(s_va, 3)
            sc.activation(
                cs[:, :], ssum[:, :], AF.Ln, scale=1.0
            ).then_inc(s_sa, 1)
            bsq(1)
            schain(bh, 3, 6)
            sc.wait_ge(s_va, 6)
            sc.activation(
                cs[:, :], ssum[:, :], AF.Ln, scale=1.0
            ).then_inc(s_sa, 1)
            sc.activation(
                invb[:, :], cs[:, :], AF.Exp, scale=-1.0
            ).then_inc(s_sa, 1)
            exps(0)
            exps(1)
            for c in range(2, NCH):
                sc.wait_ge(s_pe, c - 1)
                bsq(c)
                exps(c)

        @block.tensor
        def _(t):
            NQ = 8
            TOT = NCH * VC
            for c in range(NCH):
                k = c % 2
                t.wait_ge(s_row, c + 1)
                ia_b = i3(ia[k], bf16)
                ib_b = i3(ib[k], bf16)
                for h in range(NQ):
                    t.wait_ge(s_exp_a, NQ * c + h + 1)
                    t.wait_ge(s_exp_b, NQ * c + h + 1)
                    for vv in range(h * VC // NQ, (h + 1) * VC // NQ):
                        gv = c * VC + vv
                        bk = gv % NBANK
                        mm = t.matmul(
                            acc[bk][:, :], ia_b[:, :, vv], ib_b[:, :, vv],
                            start=(gv < NBANK), stop=(gv >= TOT - NBANK),
                        )
                        if vv == VC - 1:
                            mm.then_inc(s_pe, 1)

    return nc


def _get_nc():
    if "nc" not in _CACHE:
        _CACHE["nc"] = _build_nc()
    return _CACHE["nc"]


def run_device(a_flat, b_flat, trace=False):
    from concourse.bass_utils import run_bass_kernel_spmd

    nc = _get_nc()

    def shard(x, i):
        sl = x[i * NVOX : (i + 1) * NVOX].reshape(P, COLS)
        return np.ascontiguousarray(sl).astype(np.float16)

    in_maps = [
        {"a": shard(a_flat, i), "b": shard(b_flat, i)} for i in range(NCORES)
    ]
    kw = {}
    if trace:
        kw.update(trace=True, trace_cores=[0])
    res = run_bass_kernel_spmd(nc, in_maps, list(range(NCORES)), **kw)
    stats = np.zeros((S, S), np.float64)
    for r in res.results:
        w = np.asarray(r["stats"], np.float64)
        for i in range(NBANK):
            stats += w[:, i * S : (i + 1) * S]
    return stats, res


def finish(stats):
    n = float(NVOX_TOTAL)
    pab = stats[0:B, 0:B] / n
    pa = stats[0:B, B] / n
    pb = stats[B, 0:B] / n
    eps = 1.4e-45
    papb = np.outer(pa, pb) + eps
    mi = np.sum(pab * np.log(pab / papb + eps))
    return np.array([-mi], dtype=np.float32)


def kernel(actual, target):
    a = np.clip(np.asarray(actual, np.float32).reshape(-1), 0.0, 1.0)
    b = np.clip(np.asarray(target, np.float32).reshape(-1), 0.0, 1.0)
    stats, _ = run_device(a, b)
    return finish(stats)
